# revision 1
# baseline (speedup 1.0000x reference)
"""Causal multi-head attention (B=4, T=2048, D=1024, H=16) on 8 TRN2 NeuronCores.

Sharding: core c -> batch b = c // 2, head-group g = c % 2 (8 heads each).
Host pre-transposes x to x^T per batch and pre-slices W_qkv/W_o/biases per
head-group (1/sqrt(dh) folded into W_q/b_q on host).  Each core:

  phase 1: Q^T,K^T  (qkv^T layout, [dh, t], heads pair-stacked on partitions)
           V        (natural [t, dh] layout, ones-augmented for row-sums)
  phase 2: per head-pair, per 512-wide q-chunk, per 128-wide k-tile:
           S^T = K^T.T Q^T (two heads row-packed into one 2-bank psum tile),
           exp via ScalarE -> P^T (bf16), causal via suffix-trimming +
           triangular mask multiply on diagonal tiles,
           o^T (+rowsum) = V_aug.T @ P^T accumulated in psum,
           normalize via DVE reciprocal + rank-1 ones broadcast matmul.
  phase 3: partial out = o_norm^T.T @ W_o  ->  DRAM.

Host sums the two head-group partials per batch and adds b_o.

Matmul dtypes: fp32r (TF32-class, full PE rate at N>=256) for projections,
bf16 for score/AV stages (SBUF capacity).  A post-scheduling pass splits
multi-semaphore waits (walrus allows only one sync-wait on several ISA
structs, e.g. the fused fp32 weight-load).
"""

import sys

sys.path.insert(0, "/opt/trn_rl_repo")

import numpy as np

import concourse.bass as bass
import concourse.mybir as mybir
from concourse.bass_utils import run_bass_kernel_spmd
from concourse.tile import TileContext

F32 = mybir.dt.float32
F32R = mybir.dt.float32r
BF16 = mybir.dt.bfloat16
EXP = mybir.ActivationFunctionType.Exp

B, T, D, H = 4, 2048, 1024, 16
DH = D // H          # 64
HPC = H // 2         # heads per core = 8
DPC = HPC * DH       # 512 projected dims per core
N_CORES = 8
QC = 512             # q-chunk width in phase 2
KT = 128             # k-tile width


def split_excess_waits(nc, cap=1):
    """walrus limits sync-wait slots per ISA instruction (1 for several
    structs).  Move excess waits onto InstEventSemaphore instructions
    inserted just before the offender on the same engine."""
    n_split = 0
    for f in nc.m.functions:
        for blk in f.blocks:
            insts = blk.instructions
            out = []
            changed = False
            for inst in insts:
                si = inst.sync_info
                waits = list(si.on_wait) if si is not None else []
                if len(waits) > cap:
                    for j, w in enumerate(waits[:-cap]):
                        ev = mybir.InstEventSemaphore(
                            name=f"{inst.name}-w{j}", ins=[], outs=[]
                        )
                        ev.engine = inst.engine
                        ev.sync_info = mybir.SyncInfo(on_wait=[w], on_update=[])
                        out.append(ev)
                        n_split += 1
                    inst.sync_info = mybir.SyncInfo(
                        on_wait=waits[-cap:], on_update=list(si.on_update)
                    )
                    changed = True
                out.append(inst)
            if changed:
                blk.instructions = out
    return n_split


INST_LABELS = {}


def build():
    nc = bass.Bass(target_bir_lowering=False)

    _label = ["init"]

    def set_label(s):
        _label[0] = s

    for eng in (nc.tensor, nc.vector, nc.scalar, nc.gpsimd, nc.sync):
        orig = eng.add_instruction

        def wrapped(inst, _orig=orig):
            r = _orig(inst)
            try:
                INST_LABELS[inst.name] = _label[0]
            except Exception:
                pass
            return r

        eng.add_instruction = wrapped

    xT_d = nc.dram_tensor("xT", [D, T], F32, kind="ExternalInput")
    wqk_d = nc.dram_tensor("wqk", [D, 2 * DPC], F32, kind="ExternalInput")
    wv_d = nc.dram_tensor("wv", [D, DPC], F32, kind="ExternalInput")
    wo_d = nc.dram_tensor("wo", [DPC, D], F32, kind="ExternalInput")
    bqk_d = nc.dram_tensor("bqk", [128, 8], F32, kind="ExternalInput")
    bv_d = nc.dram_tensor("bv", [1, DPC], F32, kind="ExternalInput")
    mask_d = nc.dram_tensor("trimask", [128, 256], BF16, kind="ExternalInput")
    ones_d = nc.dram_tensor("ones", [1, 128], F32, kind="ExternalInput")
    out_d = nc.dram_tensor("out", [T, D], F32, kind="ExternalOutput")
    rsum_d = nc.dram_tensor("rsum", [32, 512], F32)  # internal scratch

    with TileContext(nc) as tc:
        with (
            tc.tile_pool(name="const", bufs=1) as constp,
            tc.tile_pool(name="wstream", bufs=2) as wp,
            tc.tile_pool(name="xt", bufs=1) as xtp,
            tc.tile_pool(name="qk", bufs=1) as qkp,
            tc.tile_pool(name="vaug", bufs=1) as vp,
            tc.tile_pool(name="onorm", bufs=1) as onp,
            tc.tile_pool(name="pt", bufs=4) as ptp,
            tc.tile_pool(name="small", bufs=4) as smallp,
            tc.tile_pool(name="osb", bufs=3) as osbp,
            tc.tile_pool(name="ps", bufs=4, space="PSUM") as psp,
            tc.tile_pool(name="spair", bufs=2, space="PSUM") as spp,
        ):
            set_label("const")
            # ---- constants ----
            wv_sb = constp.tile([128, 8, DPC], F32R, tag="wv")
            nc.sync.dma_start(
                wv_sb[:], wv_d[:].rearrange("(dt p) c -> p dt c", p=128).bitcast(F32R)
            )

            bqk_sb = constp.tile([128, 8], F32, tag="bqk")
            nc.sync.dma_start(bqk_sb[:], bqk_d[:])
            bv_sb = constp.tile([1, DPC], F32R, tag="bv")
            nc.sync.dma_start(bv_sb[:], bv_d[:].bitcast(F32R))
            mask_sb = constp.tile([128, 2, 128], BF16, tag="mask")
            nc.sync.dma_start(mask_sb[:], mask_d[:].rearrange("p (h q) -> p h q", h=2))
            ones128 = constp.tile([1, 128], F32R, tag="ones128")
            nc.sync.dma_start(ones128[:], ones_d[:].bitcast(F32R))
            ones64 = ones128[:, 0:64]

            # persistent activations
            qk_sb = [qkp.tile([128, T], BF16, tag=f"qk{j}", name=f"qk{j}") for j in range(8)]
            vaug = [vp.tile([128, HPC, DH + 1], BF16, tag=f"v{t}", name=f"v{t}") for t in range(16)]
            onorm = [onp.tile([128, T], F32R, tag=f"on{hp}", name=f"on{hp}") for hp in range(4)]

            # ---- phase 1: projections, in two t-halves ----
            def load_wj(th, j, split=False):
                w_j = wp.tile([128, 8, 128], F32R, tag="wqk", name=f"w{th}_{j}")
                wsrc = wqk_d[:, 128 * j : 128 * (j + 1)].rearrange(
                    "(dt p) c -> p dt c", p=128
                ).bitcast(F32R)
                if split:
                    nc.sync.dma_start(w_j[:, 0:2], wsrc[:, 0:2])
                    nc.sync.dma_start(w_j[:, 2:8], wsrc[:, 2:8])
                else:
                    nc.sync.dma_start(w_j[:], wsrc)
                return w_j

            for th in range(2):
                t0 = th * (T // 2)
                w_first = load_wj(th, 0, split=True) if th == 0 else None
                xt = []
                for dt in range(8):
                    x_t = xtp.tile([128, T // 2], F32R, tag=f"xt{dt}")
                    xsrc = xT_d[128 * dt : 128 * (dt + 1), t0 : t0 + T // 2].bitcast(
                        F32R
                    )
                    if th == 0:
                        nc.sync.dma_start(x_t[:, 0:512], xsrc[:, 0:512])
                        nc.sync.dma_start(x_t[:, 512:], xsrc[:, 512:])
                    else:
                        nc.sync.dma_start(x_t[:], xsrc)
                    xt.append(x_t)

                if th == 0:
                    set_label("const")
                    wv_sb = constp.tile([128, 8, DPC], F32R, tag="wv")
                    nc.sync.dma_start(
                        wv_sb[:],
                        wv_d[:].rearrange("(dt p) c -> p dt c", p=128).bitcast(F32R),
                    )

                # Q^T / K^T:  [d', t] = W[:, d'].T @ x^T
                set_label("qkv")
                for j in range(8):
                    w_j = w_first if (th == 0 and j == 0) else load_wj(th, j)
                    for tc_ in range(2):
                        ps = psp.tile([128, 512], F32, tag="ps")
                        for dt in range(8):
                            nc.tensor.matmul(
                                ps[:],
                                w_j[:, dt, :],
                                xt[dt][:, 512 * tc_ : 512 * (tc_ + 1)],
                                start=(dt == 0),
                                stop=(dt == 7),
                            )
                        nc.vector.tensor_scalar_add(
                            qk_sb[j][:, t0 + 512 * tc_ : t0 + 512 * (tc_ + 1)],
                            ps[:],
                            bqk_sb[:, j : j + 1],
                        )

                # V (natural layout), ones-augmented
                set_label("vproj")
                for tt in range(8):
                    tg = th * 8 + tt
                    ps = psp.tile([128, 512], F32, tag="ps")
                    for dt in range(8):
                        nc.tensor.matmul(
                            ps[:],
                            xt[dt][:, 128 * tt : 128 * (tt + 1)],
                            wv_sb[:, dt, :],
                            start=(dt == 0),
                            stop=False,
                        )
                    nc.tensor.matmul(
                        ps[:], ones128[:], bv_sb[:], start=False, stop=True
                    )
                    nc.vector.tensor_copy(
                        out=vaug[tg][:, :, 0:DH],
                        in_=ps[:].rearrange("p (h d) -> p h d", h=HPC),
                    )
                    nc.gpsimd.memset(vaug[tg][:, :, DH : DH + 1], 1.0)

            # ---- phase 2: attention per head pair ----
            for hp in range(4):
                qT = qk_sb[hp]
                kT = qk_sb[4 + hp]
                for c in range(4):
                    set_label(f"attn")
                    q0 = QC * c
                    ktiles = 4 * (c + 1)
                    oA = psp.tile([128, 512], F32, tag="ps")
                    oB = psp.tile([128, 512], F32, tag="ps")
                    for t in range(ktiles):
                        j = t - 4 * c
                        qs = 128 * j if j >= 0 else 0
                        sp = spp.tile([128, 1024], F32, tag="sp")
                        for half, base in ((0, 0), (1, 64)):
                            nc.tensor.matmul(
                                sp[:, 512 * half + qs : 512 * (half + 1)],
                                kT[base : base + 64, 128 * t : 128 * (t + 1)],
                                qT[base : base + 64, q0 + qs : q0 + QC],
                                start=True,
                                stop=True,
                                tile_position=(base, 0),
                            )
                        pt = ptp.tile([128, 1024], BF16, tag="pt")
                        if qs == 0:
                            nc.scalar.activation(pt[:], sp[:], EXP)
                        else:
                            spv = sp[:].rearrange("p (h q) -> p h q", h=2)[
                                :, :, qs:512
                            ]
                            ptv = pt[:].rearrange("p (h q) -> p h q", h=2)[
                                :, :, qs:512
                            ]
                            nc.scalar.activation(ptv, spv, EXP)
                        if j >= 0:
                            ptv = pt[:].rearrange("p (h q) -> p h q", h=2)[
                                :, :, qs : qs + 128
                            ]
                            nc.gpsimd.tensor_tensor(
                                ptv, ptv, mask_sb[:], mybir.AluOpType.mult
                            )
                        for o_ps, half in ((oA, 0), (oB, 1)):
                            nc.tensor.matmul(
                                o_ps[0 : DH + 1, qs:512],
                                vaug[t][:, 2 * hp + half, :],
                                pt[:, 512 * half + qs : 512 * (half + 1)],
                                start=(t == 0),
                                stop=(t == ktiles - 1),
                                skip_group_check=True,
                            )
                    # normalize both heads: reciprocal of the rowsum row from
                    # psum, broadcast across 64 partitions via a DRAM
                    # round-trip DMA (no PE involvement), then multiply
                    # straight out of the psum accumulator on DVE.
                    set_label("norm")
                    rts, bcs = [], []
                    for o_ps in (oA, oB):
                        rt = smallp.tile([1, 512], F32, tag="rt")
                        nc.vector.reciprocal(rt[:], o_ps[DH : DH + 1, :])
                        rts.append(rt)
                    for half in (0, 1):
                        ridx = (hp * 4 + c) * 2 + half
                        nc.sync.dma_start(rsum_d[ridx : ridx + 1, :], rts[half][:])
                        bc_sb = smallp.tile([64, 512], F32, tag="bc")
                        nc.sync.dma_start(
                            bc_sb[:], bass.AP(rsum_d, ridx * 512, [[0, 64], [1, 512]])
                        )
                        bcs.append(bc_sb)
                    for o_ps, base, half in ((oA, 0, 0), (oB, 64, 1)):
                        nc.vector.tensor_tensor(
                            onorm[hp][base : base + 64, q0 : q0 + QC],
                            o_ps[0:DH, :],
                            bcs[half][:],
                            mybir.AluOpType.mult,
                        )

            # ---- phase 3: output projection ----
            set_label("oproj")
            wo_sb = constp.tile([128, 4, D], F32R, tag="wo")
            nc.sync.dma_start(
                wo_sb[:], wo_d[:].rearrange("(hp p) c -> p hp c", p=128).bitcast(F32R)
            )
            for qt in range(16):
                for dc in range(2):
                    ps = psp.tile([128, 512], F32, tag="ps")
                    for hp in range(4):
                        nc.tensor.matmul(
                            ps[:],
                            onorm[hp][:, 128 * qt : 128 * (qt + 1)],
                            wo_sb[:, hp, 512 * dc : 512 * (dc + 1)],
                            start=(hp == 0),
                            stop=(hp == 3),
                        )
                    osb = osbp.tile([128, 512], F32, tag="osb")
                    nc.vector.tensor_copy(out=osb[:], in_=ps[:])
                    nc.sync.dma_start(
                        out_d[128 * qt : 128 * (qt + 1), 512 * dc : 512 * (dc + 1)],
                        osb[:],
                    )

    split_excess_waits(nc)
    return nc


TRACE = False
LAST_EXEC_NS = None

_NC = None


def _get_nc():
    global _NC
    if _NC is None:
        _NC = build()
    return _NC


def kernel(x, W_qkv, b_qkv, W_o, b_o):
    x = np.asarray(x, dtype=np.float32)
    W_qkv = np.asarray(W_qkv, dtype=np.float32)
    b_qkv = np.asarray(b_qkv, dtype=np.float32)
    W_o = np.asarray(W_o, dtype=np.float32)
    b_o = np.asarray(b_o, dtype=np.float32)
    import ml_dtypes

    scale = 1.0 / np.sqrt(np.float32(DH))

    # x^T per batch (shared between the two cores of a batch)
    xTs = [np.ascontiguousarray(x[b].T) for b in range(B)]

    # causal mask tile: keep iff q-local >= k-local (upper triangular w/ diag)
    tri1 = np.triu(np.ones((128, 128), np.float32))
    tri = np.concatenate([tri1, tri1], axis=1).astype(ml_dtypes.bfloat16)

    in_maps = []
    for c in range(N_CORES):
        b, g = divmod(c, 2)
        h0 = g * HPC
        qcols = slice(h0 * DH, h0 * DH + DPC)
        kcols = slice(D + h0 * DH, D + h0 * DH + DPC)
        vcols = slice(2 * D + h0 * DH, 2 * D + h0 * DH + DPC)
        wqk = np.concatenate(
            [W_qkv[:, qcols] * scale, W_qkv[:, kcols]], axis=1
        ).astype(np.float32)
        bqk = np.concatenate(
            [b_qkv[qcols] * scale, b_qkv[kcols]]
        ).astype(np.float32)
        in_maps.append(
            {
                "xT": xTs[b],
                "wqk": np.ascontiguousarray(wqk),
                "wv": np.ascontiguousarray(W_qkv[:, vcols]),
                "wo": np.ascontiguousarray(W_o[g * DPC : (g + 1) * DPC, :]),
                "bqk": np.ascontiguousarray(bqk.reshape(8, 128).T),
                "bv": np.ascontiguousarray(b_qkv[vcols].reshape(1, DPC)),
                "trimask": tri,
                "ones": np.ones((1, 128), np.float32),
            }
        )

    nc = _get_nc()
    global LAST_EXEC_NS
    res = None
    last_err = None
    for attempt in range(3):
        try:
            res = run_bass_kernel_spmd(
                nc, in_maps, list(range(N_CORES)), trace=TRACE
            )
            break
        except Exception as e:  # transient device wedge: retry
            last_err = e
            import time as _time

            _time.sleep(5)
    if res is None:
        raise last_err
    LAST_EXEC_NS = res.exec_time_ns
    LAST_RES = globals().setdefault("_LAST_RES", None)
    globals()["_LAST_RES"] = res
    parts = [res.results[c]["out"] for c in range(N_CORES)]
    out = np.empty((B, T, D), np.float32)
    for b in range(B):
        out[b] = parts[2 * b] + parts[2 * b + 1] + b_o[None, :]
    return out



# revision 14
# speedup vs baseline: 1.0774x; 1.0774x over previous
"""Causal multi-head attention (B=4, T=2048, D=1024, H=16) on 8 TRN2 NeuronCores.

Sharding: core c -> batch b = c // 2, head-group g = c % 2 (8 heads each).
Host pre-transposes x to x^T per batch, converts everything to bf16, and
pre-slices W_qkv/W_o/biases per head-group (1/sqrt(dh)=1/8 folded into
W_q/b_q exactly).  Each core:

  phase 1 (two T-halves, interleaved with attention):
      Q^T,K^T  (qkv^T layout, [dh, t], heads pair-stacked on partitions)
      V        (natural [t, dh] layout, ones-augmented for row-sums)
      all matmuls bf16 (fp32r runs at ~half PE rate; bf16 is full rate).
  phase 2: attention, q-chunk-outer / head-pair-inner so chunks 0,1 (which
      only need the first T/2 of K/V/Q) start right after t-half 0:
      S^T = K^T.T Q^T (two heads row-packed, concurrent via tile_position),
      off-diagonal k-tiles: exp on ScalarE -> P^T (bf16);
      diagonal k-tiles: fused exp+causal-mask on DVE via the Schraudolph
      bit-trick: i16 = sat(round(S*A + Bmask)), Bmask = B on keep and
      B - 1e9 on masked lanes so the int16 saturates to -32768 = bf16 -0.0;
      o^T (+rowsum) = V_aug.T @ P^T accumulated in psum; raw o^T copied to
      SBUF (psum freed fast), row-sum reciprocals batched per chunk on DVE
      ([8,512] in one instruction), broadcast via a DRAM round-trip DMA,
      normalize on DVE.
  phase 3: output projection per q-chunk, interleaved one chunk behind
      attention so the normalize pipeline is off the critical path.

Host sums the two head-group partials per batch and adds b_o.
"""

import sys

sys.path.insert(0, "/opt/trn_rl_repo")

import numpy as np

import concourse.bass as bass
import concourse.mybir as mybir
from concourse.bass_utils import run_bass_kernel_spmd
from concourse.tile import TileContext

F32 = mybir.dt.float32
BF16 = mybir.dt.bfloat16
I16 = mybir.dt.int16
U16 = mybir.dt.uint16
EXP = mybir.ActivationFunctionType.Exp
MULT = mybir.AluOpType.mult
ADD = mybir.AluOpType.add

B, T, D, H = 4, 2048, 1024, 16
DH = D // H          # 64
HPC = H // 2         # heads per core = 8
DPC = HPC * DH       # 512 projected dims per core
N_CORES = 8
QC = 512             # q-chunk width
KT = 128             # k-tile width

SCH_A = 184.66496030     # 128 * log2(e)
SCH_B = 16256.0 - 7.4    # 127*128 with mean-centering correction
SCH_MASKED = SCH_B - 1.0e9   # saturates int16 -> -32768 -> bf16 -0.0


def split_excess_waits(nc, cap=1):
    """walrus limits sync-wait slots per ISA instruction (1 for several
    structs).  Move excess waits onto InstEventSemaphore instructions
    inserted just before the offender on the same engine."""
    n_split = 0
    for f in nc.m.functions:
        for blk in f.blocks:
            insts = blk.instructions
            out = []
            changed = False
            for inst in insts:
                si = inst.sync_info
                waits = list(si.on_wait) if si is not None else []
                if len(waits) > cap:
                    for j, w in enumerate(waits[:-cap]):
                        ev = mybir.InstEventSemaphore(
                            name=f"{inst.name}-w{j}", ins=[], outs=[]
                        )
                        ev.engine = inst.engine
                        ev.sync_info = mybir.SyncInfo(on_wait=[w], on_update=[])
                        out.append(ev)
                        n_split += 1
                    inst.sync_info = mybir.SyncInfo(
                        on_wait=waits[-cap:], on_update=list(si.on_update)
                    )
                    changed = True
                out.append(inst)
            if changed:
                blk.instructions = out
    return n_split


INST_LABELS = {}


def build():
    nc = bass.Bass(target_bir_lowering=False)

    _label = ["init"]

    def set_label(s):
        _label[0] = s

    for eng in (nc.tensor, nc.vector, nc.scalar, nc.gpsimd, nc.sync):
        orig = eng.add_instruction

        def wrapped(inst, _orig=orig):
            r = _orig(inst)
            try:
                INST_LABELS[inst.name] = _label[0]
            except Exception:
                pass
            return r

        eng.add_instruction = wrapped

    xT_d = nc.dram_tensor("xT", [D, T], BF16, kind="ExternalInput")
    wqk_d = nc.dram_tensor("wqk", [D, 2 * DPC], BF16, kind="ExternalInput")
    wv_d = nc.dram_tensor("wv", [D, DPC], BF16, kind="ExternalInput")
    wo_d = nc.dram_tensor("wo", [DPC, D], BF16, kind="ExternalInput")
    bqk_d = nc.dram_tensor("bqk", [128, 8], F32, kind="ExternalInput")
    bv_d = nc.dram_tensor("bv", [1, DPC], BF16, kind="ExternalInput")
    bmask_d = nc.dram_tensor("bmask", [128, 1024], F32, kind="ExternalInput")
    ones_d = nc.dram_tensor("ones", [1, 128], BF16, kind="ExternalInput")
    out_d = nc.dram_tensor("out", [T, D], F32, kind="ExternalOutput")
    rsraw_d = nc.dram_tensor("rsraw", [4, 4096], BF16)  # raw rowsums / chunk
    rrec_d = nc.dram_tensor("rrec", [4, 4096], BF16)    # reciprocals / chunk

    with TileContext(nc) as tc:
        with (
            tc.tile_pool(name="const", bufs=1) as constp,
            tc.tile_pool(name="wstream", bufs=2) as wp,
            tc.tile_pool(name="xt", bufs=2) as xtp,
            tc.tile_pool(name="qk", bufs=1) as qkp,
            tc.tile_pool(name="vaug", bufs=1) as vp,
            tc.tile_pool(name="onorm", bufs=1) as onp,
            tc.tile_pool(name="pt", bufs=4) as ptp,
            tc.tile_pool(name="sc", bufs=4) as scp,
            tc.tile_pool(name="rs", bufs=4) as rsp,
            tc.tile_pool(name="bc", bufs=4) as bcp,
            tc.tile_pool(name="osb", bufs=3) as osbp,
            tc.tile_pool(name="ps", bufs=4, space="PSUM") as psp,
            tc.tile_pool(name="spair", bufs=2, space="PSUM") as spp,
        ):
            set_label("const")
            # ---- constants ----
            bqk_sb = constp.tile([128, 8], F32, tag="bqk")
            nc.sync.dma_start(bqk_sb[:], bqk_d[:])
            bv_sb = constp.tile([1, DPC], BF16, tag="bv")
            nc.sync.dma_start(bv_sb[:], bv_d[:])
            bmask_sb = constp.tile([128, 2, 512], F32, tag="bmask")
            nc.sync.dma_start(
                bmask_sb[:], bmask_d[:].rearrange("p (h q) -> p h q", h=2)
            )
            ones128 = constp.tile([1, 128], BF16, tag="ones128")
            nc.sync.dma_start(ones128[:], ones_d[:])

            # persistent activations
            qk_sb = [
                qkp.tile([128, T], BF16, tag=f"qk{j}", name=f"qk{j}")
                for j in range(8)
            ]
            vaug = [
                vp.tile([128, HPC, DH + 1], BF16, tag=f"v{t}", name=f"v{t}")
                for t in range(16)
            ]
            onorm = [
                onp.tile([128, T], BF16, tag=f"on{hp}", name=f"on{hp}")
                for hp in range(4)
            ]

            # ---- phase 1: projections for one t-half ----
            def load_wj(th, j, split=False):
                w_j = wp.tile([128, 8, 128], BF16, tag="wqk", name=f"w{th}_{j}")
                wsrc = wqk_d[:, 128 * j : 128 * (j + 1)].rearrange(
                    "(dt p) c -> p dt c", p=128
                )
                if split:
                    nc.sync.dma_start(w_j[:, 0:2], wsrc[:, 0:2])
                    nc.sync.dma_start(w_j[:, 2:8], wsrc[:, 2:8])
                else:
                    nc.sync.dma_start(w_j[:], wsrc)
                return w_j

            wv_sb = None

            def proj_half(th):
                nonlocal wv_sb
                t0 = th * (T // 2)
                set_label("qkv")
                w_first = load_wj(th, 0, split=True) if th == 0 else None
                xt = []
                for dt in range(8):
                    x_t = xtp.tile([128, T // 2], BF16, tag=f"xt{dt}")
                    xsrc = xT_d[128 * dt : 128 * (dt + 1), t0 : t0 + T // 2]
                    if th == 0:
                        nc.sync.dma_start(x_t[:, 0:512], xsrc[:, 0:512])
                        nc.sync.dma_start(x_t[:, 512:], xsrc[:, 512:])
                    else:
                        nc.sync.dma_start(x_t[:], xsrc)
                    xt.append(x_t)

                if th == 0:
                    set_label("const")
                    wv_sb = constp.tile([128, 8, DPC], BF16, tag="wv")
                    nc.sync.dma_start(
                        wv_sb[:], wv_d[:].rearrange("(dt p) c -> p dt c", p=128)
                    )

                # Q^T / K^T:  [d', t] = W[:, d'].T @ x^T
                set_label("qkv")
                for j in range(8):
                    w_j = w_first if (th == 0 and j == 0) else load_wj(th, j)
                    for tc_ in range(2):
                        ps = psp.tile([128, 512], F32, tag="ps")
                        for dt in range(8):
                            nc.tensor.matmul(
                                ps[:],
                                w_j[:, dt, :],
                                xt[dt][:, 512 * tc_ : 512 * (tc_ + 1)],
                                start=(dt == 0),
                                stop=(dt == 7),
                            )
                        nc.vector.tensor_scalar_add(
                            qk_sb[j][:, t0 + 512 * tc_ : t0 + 512 * (tc_ + 1)],
                            ps[:],
                            bqk_sb[:, j : j + 1],
                        )

                # V (natural layout), ones-augmented
                set_label("vproj")
                for tt in range(8):
                    tg = th * 8 + tt
                    ps = psp.tile([128, 512], F32, tag="ps")
                    for dt in range(8):
                        nc.tensor.matmul(
                            ps[:],
                            xt[dt][:, 128 * tt : 128 * (tt + 1)],
                            wv_sb[:, dt, :],
                            start=(dt == 0),
                            stop=False,
                        )
                    nc.tensor.matmul(
                        ps[:], ones128[:], bv_sb[:], start=False, stop=True
                    )
                    nc.vector.tensor_copy(
                        out=vaug[tg][:, :, 0:DH],
                        in_=ps[:].rearrange("p (h d) -> p h d", h=HPC),
                    )
                    nc.gpsimd.memset(vaug[tg][:, :, DH : DH + 1], 1.0)

            # ---- phase 2: one attention (hp, c) block ----
            def attn_block(c, hp):
                set_label("attn")
                qT = qk_sb[hp]
                kT = qk_sb[4 + hp]
                q0 = QC * c
                ktiles = 4 * (c + 1)
                oA = psp.tile([128, 512], F32, tag="ps")
                oB = psp.tile([128, 512], F32, tag="ps")
                for t in range(ktiles):
                    j = t - 4 * c
                    qs = 128 * j if j >= 0 else 0
                    sp = spp.tile([128, 1024], F32, tag="sp")
                    for half, base in ((0, 0), (1, 64)):
                        nc.tensor.matmul(
                            sp[:, 512 * half + qs : 512 * (half + 1)],
                            kT[base : base + 64, 128 * t : 128 * (t + 1)],
                            qT[base : base + 64, q0 + qs : q0 + QC],
                            start=True,
                            stop=True,
                            tile_position=(base, 0),
                        )
                    pt = ptp.tile([128, 1024], BF16, tag="pt")
                    if j < 0:
                        nc.scalar.activation(pt[:], sp[:], EXP)
                    else:
                        spv = sp[:].rearrange("p (h q) -> p h q", h=2)[
                            :, :, qs:512
                        ]
                        ptv = pt[:].rearrange("p (h q) -> p h q", h=2)[
                            :, :, qs:512
                        ]
                        nc.vector.scalar_tensor_tensor(
                            ptv.bitcast(I16),
                            spv,
                            SCH_A,
                            bmask_sb[:, :, 0 : 512 - qs],
                            MULT,
                            ADD,
                        )
                    for o_ps, half in ((oA, 0), (oB, 1)):
                        nc.tensor.matmul(
                            o_ps[0 : DH + 1, qs:512],
                            vaug[t][:, 2 * hp + half, :],
                            pt[:, 512 * half + qs : 512 * (half + 1)],
                            start=(t == 0),
                            stop=(t == ktiles - 1),
                            skip_group_check=True,
                        )
                # evict raw o + rowsums from psum; rowsum rows go straight
                # to DRAM (per-row [1,512] tiles keep partition base 0)
                set_label("norm")
                sc = scp.tile([128, 512], BF16, tag="sc")
                for o_ps, half in ((oA, 0), (oB, 1)):
                    idx = 2 * hp + half
                    rrow = rsp.tile([1, 512], BF16, tag="rrow")
                    nc.vector.tensor_copy(
                        out=rrow[:], in_=o_ps[DH : DH + 1, :]
                    )
                    nc.gpsimd.dma_start(
                        rsraw_d[c : c + 1, idx * 512 : (idx + 1) * 512],
                        rrow[:],
                    )
                    nc.vector.tensor_copy(
                        out=sc[64 * half : 64 * half + 64, :],
                        in_=o_ps[0:DH, :],
                    )
                return sc

            def attn_chunk(c):
                scs = []
                for hp in range(4):
                    scs.append(attn_block(c, hp))
                # batched reciprocal: reload the chunk's 8 rowsum rows
                # respread as [128, 32] so the per-lane-serial reciprocal
                # touches only 32 elements per lane
                set_label("norm")
                rload = rsp.tile([128, 32], BF16, tag="rload")
                nc.gpsimd.dma_start(
                    rload[:], bass.AP(rsraw_d, c * 4096, [[32, 128], [1, 32]])
                )
                rrec = rsp.tile([128, 32], BF16, tag="rrec")
                with nc.allow_low_precision(
                    reason="bf16 softmax-normalizer reciprocal; 0.4% rel "
                    "is far inside the output tolerance"
                ):
                    nc.vector.reciprocal(rrec[:], rload[:])
                nc.gpsimd.dma_start(
                    bass.AP(rrec_d, c * 4096, [[32, 128], [1, 32]]), rrec[:]
                )
                for hp in range(4):
                    # both halves' recip rows in one [128,512] tile so the
                    # normalize mult's SBUF inputs share base partitions
                    bc = bcp.tile([128, 512], BF16, tag="bc")
                    nc.gpsimd.dma_start(
                        bc[:],
                        bass.AP(
                            rrec_d,
                            c * 4096 + 2 * hp * 512,
                            [[512, 2], [0, 64], [1, 512]],
                        ),
                    )
                    for half in range(2):
                        nc.vector.tensor_tensor(
                            onorm[hp][64 * half : 64 * half + 64, QC * c : QC * (c + 1)],
                            scs[hp][64 * half : 64 * half + 64, :],
                            bc[64 * half : 64 * half + 64, :],
                            MULT,
                        )

            # ---- phase 3: output projection for one q-chunk ----
            wo_sb = None

            def oproj_chunk(c):
                nonlocal wo_sb
                set_label("oproj")
                if wo_sb is None:
                    wo_sb = constp.tile([128, 4, D], BF16, tag="wo")
                    nc.sync.dma_start(
                        wo_sb[:], wo_d[:].rearrange("(hp p) c -> p hp c", p=128)
                    )
                for qt in range(4 * c, 4 * c + 4):
                    for dc in range(2):
                        ps = psp.tile([128, 512], F32, tag="ps")
                        for hp in range(4):
                            nc.tensor.matmul(
                                ps[:],
                                onorm[hp][:, 128 * qt : 128 * (qt + 1)],
                                wo_sb[:, hp, 512 * dc : 512 * (dc + 1)],
                                start=(hp == 0),
                                stop=(hp == 3),
                            )
                        osb = osbp.tile([128, 512], F32, tag="osb")
                        nc.scalar.activation(
                            osb[:], ps[:], mybir.ActivationFunctionType.Copy
                        )
                        nc.sync.dma_start(
                            out_d[
                                128 * qt : 128 * (qt + 1),
                                512 * dc : 512 * (dc + 1),
                            ],
                            osb[:],
                        )

            # ---- emission order: overlap t-half-0 projections with early
            # attention chunks; oproj trails attention by one chunk ----
            proj_half(0)
            attn_chunk(0)
            attn_chunk(1)
            oproj_chunk(0)
            proj_half(1)
            oproj_chunk(1)
            attn_chunk(2)
            attn_chunk(3)
            oproj_chunk(2)
            oproj_chunk(3)

    split_excess_waits(nc)
    return nc


TRACE = False
LAST_EXEC_NS = None

_NC = None


def _get_nc():
    global _NC
    if _NC is None:
        _NC = build()
    return _NC


def kernel(x, W_qkv, b_qkv, W_o, b_o):
    x = np.asarray(x, dtype=np.float32)
    W_qkv = np.asarray(W_qkv, dtype=np.float32)
    b_qkv = np.asarray(b_qkv, dtype=np.float32)
    W_o = np.asarray(W_o, dtype=np.float32)
    b_o = np.asarray(b_o, dtype=np.float32)
    import ml_dtypes

    BF = ml_dtypes.bfloat16
    scale = 1.0 / np.sqrt(np.float32(DH))  # = 0.125, exact in bf16

    # x^T per batch (shared between the two cores of a batch)
    xTs = [np.ascontiguousarray(x[b].T.astype(BF)) for b in range(B)]

    # Bmask [128, 2 heads, 512]: for the k-tile suffix starting at the
    # diagonal block: first 128 cols triangular (keep iff q >= k), rest keep.
    bm1 = np.full((128, 512), np.float32(SCH_B), np.float32)
    qq = np.arange(128)[None, :]
    pp = np.arange(128)[:, None]
    # blk[k, q]: keep iff q >= k (upper triangular incl. diagonal)
    blk = np.where(qq >= pp, np.float32(SCH_B), np.float32(SCH_MASKED))
    bm1[:, 0:128] = blk
    bmask = np.concatenate([bm1, bm1], axis=1).astype(np.float32)

    in_maps = []
    for c in range(N_CORES):
        b, g = divmod(c, 2)
        h0 = g * HPC
        qcols = slice(h0 * DH, h0 * DH + DPC)
        kcols = slice(D + h0 * DH, D + h0 * DH + DPC)
        vcols = slice(2 * D + h0 * DH, 2 * D + h0 * DH + DPC)
        wqk = np.concatenate(
            [W_qkv[:, qcols] * scale, W_qkv[:, kcols]], axis=1
        ).astype(BF)
        bqk = np.concatenate(
            [b_qkv[qcols] * scale, b_qkv[kcols]]
        ).astype(np.float32)
        in_maps.append(
            {
                "xT": xTs[b],
                "wqk": np.ascontiguousarray(wqk),
                "wv": np.ascontiguousarray(W_qkv[:, vcols].astype(BF)),
                "wo": np.ascontiguousarray(W_o[g * DPC : (g + 1) * DPC, :].astype(BF)),
                "bqk": np.ascontiguousarray(bqk.reshape(8, 128).T),
                "bv": np.ascontiguousarray(b_qkv[vcols].astype(BF).reshape(1, DPC)),
                "bmask": bmask,
                "ones": np.ones((1, 128), BF),
            }
        )

    nc = _get_nc()
    global LAST_EXEC_NS
    res = None
    last_err = None
    for attempt in range(3):
        try:
            res = run_bass_kernel_spmd(
                nc, in_maps, list(range(N_CORES)), trace=TRACE
            )
            break
        except Exception as e:  # transient device wedge: retry
            last_err = e
            import time as _time

            _time.sleep(5)
    if res is None:
        raise last_err
    LAST_EXEC_NS = res.exec_time_ns
    globals()["_LAST_RES"] = res
    parts = [res.results[c]["out"] for c in range(N_CORES)]
    out = np.empty((B, T, D), np.float32)
    for b in range(B):
        out[b] = parts[2 * b] + parts[2 * b + 1] + b_o[None, :]
    return out
